# revision 6
# baseline (speedup 1.0000x reference)
"""Trainium2 Bass kernel: quantized Conformer FFN block (LN -> int8-dequant
fc1 -> SiLU -> int8-dequant fc2 -> residual), data-parallel over batch on 8
NeuronCores.

Per-core math (core c handles batch c, 2048 tokens):
  h  = LayerNorm(x) * gamma + beta                     (token-major, fp32)
  hT = transpose(h) per 128x128 block via PE           (feature-major, bf16)
  w1 = dequant(fc1_q, fc1_scales) in bf16              (resident in SBUF)
  yT = silu(w1.T @ hT + b1)                            (feature-major, bf16)
  w2 = 0.5 * dequant(fc2_q, fc2_scales) in bf16        (DRAM scratch, streamed)
  z  = yT.T @ w2  (+0.5*b2 folded into residual base)  (token-major, fp32)
  out = x + 0.5*b2 + z

All dequantization happens on-device.  Scale rows are broadcast across
partitions with K=1 ones-matmuls on the PE (out[p,n] = ones[0,p]*s[0,n]).
The 0.5 output scale is folded into the w2 dequant (ones tile = 0.5) and the
fc2 bias (b2 scaled by 0.5 on-device).
"""

import numpy as np

import concourse.bass as bass
import concourse.tile as tile
from concourse import bacc, mybir
from concourse.bass_utils import run_bass_kernel_spmd
from concourse.masks import make_identity

F32 = mybir.dt.float32
BF16 = mybir.dt.bfloat16
I32 = mybir.dt.int32
U8 = mybir.dt.uint8
AF = mybir.ActivationFunctionType
OP = mybir.AluOpType

EPS = 1e-5
N_CORES = 8
B, S, H, F, G = 8, 2048, 1024, 4096, 128

FULL_CFG = dict(TOK=S, H=H, F=F, SLAB=512)


def _chunks(total, size):
    out = []
    o = 0
    while o < total:
        c = min(size, total - o)
        out.append((o, c))
        o += c
    return out


def build_ffn(tc, out_ap, ins, cfg):
    nc = tc.nc
    TOK, HH, FF, SLAB = cfg["TOK"], cfg["H"], cfg["F"], cfg["SLAB"]
    KG1 = HH // G          # fc1 k-groups (H partition tiles)
    KG2 = FF // G          # fc2 k-groups (F partition tiles)
    NF = FF // 128         # fc1 output feature tiles
    NSLAB = TOK // SLAB
    TPS = SLAB // 128      # token tiles per slab
    FC1_CH = _chunks(FF, 512)   # dequant/broadcast chunks over F
    FC2_CH = _chunks(HH, 512)   # fc2 output chunks over H
    BN_FMAX = min(512, HH)
    NSUB = HH // BN_FMAX

    x_d = ins["x"]
    q1_d, s1_d, b1_d = ins["fc1_q"], ins["fc1_scales"], ins["fc1_bias"]
    q2_d, s2_d, b2_d = ins["fc2_q"], ins["fc2_scales"], ins["fc2_bias"]
    g_d, be_d = ins["ln_gamma"], ins["ln_beta"]

    const = tc.alloc_tile_pool(name="const", bufs=1)
    w1pool = tc.alloc_tile_pool(name="w1", bufs=KG1)
    spool = tc.alloc_tile_pool(name="srow", bufs=2)
    qpool = tc.alloc_tile_pool(name="qraw", bufs=3)
    fpool = tc.alloc_tile_pool(name="qflt", bufs=3)
    w2tmp = tc.alloc_tile_pool(name="w2tmp", bufs=3)
    xpool = tc.alloc_tile_pool(name="xres", bufs=2 * TPS + 2)
    stpool = tc.alloc_tile_pool(name="stats", bufs=6)
    hpool = tc.alloc_tile_pool(name="hnorm", bufs=3)
    htpool = tc.alloc_tile_pool(name="hT", bufs=KG1 + 2)
    ypool = tc.alloc_tile_pool(name="yT", bufs=NF + 2)
    w2rpool = tc.alloc_tile_pool(name="w2r", bufs=6)
    opool = tc.alloc_tile_pool(name="outs", bufs=4)
    dpool = tc.alloc_tile_pool(name="dram", bufs=1, space="DRAM")
    ps_fc1 = tc.alloc_tile_pool(name="psfc1", bufs=2, space="PSUM")
    ps_fc2 = tc.alloc_tile_pool(name="psfc2", bufs=TPS, space="PSUM")
    ps_aux = tc.alloc_tile_pool(name="psaux", bufs=2, space="PSUM")

    # ---- constants -------------------------------------------------------
    ones1 = const.tile([1, 128], F32)
    nc.vector.memset(ones1, 1.0)
    oneshalf = const.tile([1, 128], F32)
    nc.vector.memset(oneshalf, 0.5)
    ident = const.tile([128, 128], F32)
    make_identity(nc, ident)
    eps_t = const.tile([128, 1], F32)
    nc.vector.memset(eps_t, EPS)

    def load_transposed_vec(vec_ap, n, name):
        """[n*128] DRAM vector -> [128, n] per-partition SBUF tile."""
        stage = stpool.tile([n, 128], F32, tag="vstage", name=f"{name}_stage")
        nc.sync.dma_start(out=stage, in_=vec_ap.rearrange("(a b) -> a b", a=n))
        pt = ps_aux.tile([128, n], F32, tag="aux", name=f"{name}_ps")
        nc.tensor.transpose(pt, stage, ident[0:n, 0:n])
        dst = const.tile([128, n], F32, name=f"{name}T")
        nc.scalar.copy(out=dst, in_=pt)
        return dst

    b1T = load_transposed_vec(b1_d, NF, "b1")       # fc1 bias per F-tile
    gT = load_transposed_vec(g_d, KG1, "gam")       # ln gamma per H-group
    beT = load_transposed_vec(be_d, KG1, "bet")     # ln beta per H-group

    # residual base: 0.5*fc2_bias broadcast to all partitions
    b2h = const.tile([128, HH], F32)
    b2_bcast = bass.AP(tensor=b2_d.tensor, offset=b2_d.offset,
                       ap=[[0, 128]] + list(b2_d.ap))
    nc.gpsimd.dma_start(out=b2h, in_=b2_bcast)
    nc.scalar.mul(out=b2h, in_=b2h, mul=0.5)

    # ---- dequant w1 (column-major so fc1 can start early) ---------------
    w1_t = []
    for g in range(KG1):
        w1g = w1pool.tile([128, FF], BF16, tag="w1t", name=f"w1_{g}")
        w1_t.append(w1g)
    for (co, cw) in FC1_CH:
        for g in range(KG1):
            srow = spool.tile([1, 512], F32, tag="srow", name="srow1")
            nc.sync.dma_start(out=srow[:, :cw], in_=s1_d[g : g + 1, co : co + cw])
            pbc = ps_aux.tile([128, 512], F32, tag="aux", name="pbc1")
            nc.tensor.matmul(pbc[:, :cw], lhsT=ones1, rhs=srow[:, :cw],
                             start=True, stop=True)
            qt = qpool.tile([128, 512], U8, tag="qraw", name="q1t")
            nc.sync.dma_start(out=qt[:, :cw],
                              in_=q1_d[g * 128 : (g + 1) * 128, co : co + cw])
            qf = fpool.tile([128, 512], F32, tag="qflt", name="q1f")
            nc.scalar.activation(out=qf[:, :cw], in_=qt[:, :cw], func=AF.Copy,
                                 bias=-128.0, scale=1.0)
            nc.vector.tensor_tensor(out=w1_t[g][:, co : co + cw], in0=qf[:, :cw],
                                    in1=pbc[:, :cw], op=OP.mult)

    # ---- dequant w2 -> DRAM scratch (x0.5 folded via oneshalf) ----------
    w2s = dpool.tile([FF, HH], BF16)
    for (co, cw) in FC2_CH:
        for k in range(KG2):
            srow = spool.tile([1, 512], F32, tag="srow", name="srow2")
            nc.sync.dma_start(out=srow[:, :cw], in_=s2_d[k : k + 1, co : co + cw])
            pbc = ps_aux.tile([128, 512], F32, tag="aux", name="pbc2")
            nc.tensor.matmul(pbc[:, :cw], lhsT=oneshalf, rhs=srow[:, :cw],
                             start=True, stop=True)
            qt = qpool.tile([128, 512], U8, tag="qraw", name="q2t")
            nc.sync.dma_start(out=qt[:, :cw],
                              in_=q2_d[k * 128 : (k + 1) * 128, co : co + cw])
            qf = fpool.tile([128, 512], F32, tag="qflt", name="q2f")
            nc.scalar.activation(out=qf[:, :cw], in_=qt[:, :cw], func=AF.Copy,
                                 bias=-128.0, scale=1.0)
            wt = w2tmp.tile([128, 512], BF16, tag="w2tmp", name="w2c")
            nc.vector.tensor_tensor(out=wt[:, :cw], in0=qf[:, :cw],
                                    in1=pbc[:, :cw], op=OP.mult)
            nc.sync.dma_start(out=w2s[k * 128 : (k + 1) * 128, co : co + cw],
                              in_=wt[:, :cw])

    # ---- main token loop -------------------------------------------------
    for s in range(NSLAB):
        row0 = s * SLAB
        # LayerNorm + transpose to feature-major bf16
        hT_t = []
        for g in range(KG1):
            htg = htpool.tile([128, SLAB], BF16, tag="hT", name=f"hT_{g}")
            hT_t.append(htg)
        x_t = []
        for t in range(TPS):
            r = row0 + t * 128
            xt = xpool.tile([128, HH], F32, tag="xres", name="xt")
            nc.sync.dma_start(out=xt, in_=x_d[r : r + 128, :])
            x_t.append(xt)
            stats = stpool.tile([128, NSUB, 6], F32, tag="bst", name="stats")
            for i in range(NSUB):
                nc.vector.bn_stats(out=stats[:, i, :],
                                   in_=xt[:, i * BN_FMAX : (i + 1) * BN_FMAX])
            mv = stpool.tile([128, 2], F32, tag="bmv", name="mv")
            nc.vector.bn_aggr(out=mv, in_=stats)
            rstd = stpool.tile([128, 1], F32, tag="brs", name="rstd")
            nc.scalar.activation(out=rstd, in_=mv[:, 1:2], func=AF.Sqrt,
                                 bias=eps_t, scale=1.0)
            nc.vector.reciprocal(out=rstd, in_=rstd)
            ht = hpool.tile([128, HH], F32, tag="hnorm", name="ht")
            nc.vector.tensor_scalar(out=ht, in0=xt, scalar1=mv[:, 0:1],
                                    scalar2=rstd, op0=OP.subtract, op1=OP.mult)
            # x tile becomes the residual base: x + 0.5*b2
            nc.vector.tensor_tensor(out=xt, in0=xt, in1=b2h, op=OP.add)
            for g in range(KG1):
                ptp = ps_aux.tile([128, 512], F32, tag="aux", name="ptp")
                nc.tensor.transpose(ptp[:, 0:128],
                                    ht[:, g * 128 : (g + 1) * 128], ident)
                # gamma/beta fold: hT = psum*gamma[g] + beta[g]  (per-partition)
                nc.scalar.activation(out=hT_t[g][:, t * 128 : (t + 1) * 128],
                                     in_=ptp[:, 0:128], func=AF.Identity,
                                     bias=beT[:, g : g + 1],
                                     scale=gT[:, g : g + 1])

        # fc1: yT[f] = silu(w1[:, f].T @ hT + b1[f])
        yT_t = []
        for f in range(NF):
            p1 = ps_fc1.tile([128, SLAB], F32, tag="p1", name="p1")
            for g in range(KG1):
                nc.tensor.matmul(p1, lhsT=w1_t[g][:, f * 128 : (f + 1) * 128],
                                 rhs=hT_t[g], start=(g == 0), stop=(g == KG1 - 1))
            yt = ypool.tile([128, SLAB], BF16, tag="yT", name=f"yT_{f}")
            if cfg.get("silu_kind", "act") == "act":
                nc.scalar.activation(out=yt, in_=p1, func=AF.Silu,
                                     bias=b1T[:, f : f + 1], scale=1.0)
            else:
                # CoreSim has no Silu: y = v*sigmoid(v), v = psum + bias
                v = hpool.tile([128, SLAB], F32, tag="vsilu", name="vsilu")
                nc.scalar.activation(out=v, in_=p1, func=AF.Identity,
                                     bias=b1T[:, f : f + 1], scale=1.0)
                sg = hpool.tile([128, SLAB], F32, tag="sgsilu", name="sgsilu")
                nc.scalar.activation(out=sg, in_=v, func=AF.Sigmoid)
                nc.vector.tensor_tensor(out=yt, in0=v, in1=sg, op=OP.mult)
            yT_t.append(yt)

        # fc2: z[m] = yT.T @ w2  (token-major out), + residual, -> DRAM
        for (co, cw) in FC2_CH:
            pz = []
            for m in range(TPS):
                pzm = ps_fc2.tile([128, 512], F32, tag="pz", name=f"pz_{m}")
                pz.append(pzm)
            for k in range(KG2):
                w2r = w2rpool.tile([128, 512], BF16, tag="w2r", name="w2r")
                nc.sync.dma_start(out=w2r[:, :cw],
                                  in_=w2s[k * 128 : (k + 1) * 128, co : co + cw])
                for m in range(TPS):
                    nc.tensor.matmul(pz[m][:, :cw],
                                     lhsT=yT_t[k][:, m * 128 : (m + 1) * 128],
                                     rhs=w2r[:, :cw], start=(k == 0),
                                     stop=(k == KG2 - 1))
            for m in range(TPS):
                ot = opool.tile([128, 512], F32, tag="outs", name="ot")
                nc.vector.tensor_tensor(out=ot[:, :cw], in0=pz[m][:, :cw],
                                        in1=x_t[m][:, co : co + cw], op=OP.add)
                r = row0 + m * 128
                nc.sync.dma_start(out=out_ap[r : r + 128, co : co + cw],
                                  in_=ot[:, :cw])

    for p in reversed((const, w1pool, spool, qpool, fpool, w2tmp, xpool,
                       stpool, hpool, htpool, ypool, w2rpool, opool, dpool,
                       ps_fc1, ps_fc2, ps_aux)):
        p.release()


_CACHE = {}

_INPUT_SPECS = [
    ("x", (S, H), np.float32),
    ("ln_gamma", (H,), np.float32),
    ("ln_beta", (H,), np.float32),
    ("fc1_q", (H, F), np.uint8),
    ("fc1_scales", (H // G, F), np.float32),
    ("fc1_bias", (F,), np.float32),
    ("fc2_q", (F, H), np.uint8),
    ("fc2_scales", (F // G, H), np.float32),
    ("fc2_bias", (H,), np.float32),
]

_NP2MY = {np.float32: F32, np.int32: I32, np.uint8: U8}


def _get_compiled():
    if "nc" not in _CACHE:
        nc = bacc.Bacc("TRN2", target_bir_lowering=False, debug=False,
                       enable_asserts=False, num_devices=N_CORES)
        ins = {}
        for name, shape, dt in _INPUT_SPECS:
            ins[name] = nc.dram_tensor(name, list(shape), _NP2MY[dt],
                                       kind="ExternalInput").ap()
        out_ap = nc.dram_tensor("out", [S, H], F32, kind="ExternalOutput").ap()
        with tile.TileContext(nc) as tc:
            build_ffn(tc, out_ap, ins, FULL_CFG)
        nc.compile()
        _CACHE["nc"] = nc
    return _CACHE["nc"]


def _run(inputs, **kwargs):
    nc = _get_compiled()
    arrs = {name: np.ascontiguousarray(inputs[name], dtype=dt)
            for name, _, dt in _INPUT_SPECS if name != "x"}
    x = np.ascontiguousarray(inputs["x"], dtype=np.float32)
    in_maps = [dict(arrs, x=x[c]) for c in range(N_CORES)]
    return run_bass_kernel_spmd(nc, in_maps, core_ids=list(range(N_CORES)),
                                **kwargs)


def kernel(**inputs):
    res = _run(inputs)
    out = np.stack([res.results[c]["out"] for c in range(N_CORES)], axis=0)
    return out.astype(np.float32, copy=False)
